# revision 11
# baseline (speedup 1.0000x reference)
"""CRF log-likelihood kernel for Trainium2 (Bass/Tile), 8-core data parallel.

out[b] = gold_path_score(b) - logZ(b)

logZ via K=16 parallel forward chains in the exp domain. Chain k owns
el-times (32k, 32k+32] and starts BURN=8 ticks early at a_k = 32k-8 with an
arbitrary positive init (the el slice at a_k). Products of positive matrices
contract in the Hilbert projective metric (Birkhoff coefficient of
E = exp(trans) is <= tanh(1/2) ~ 0.46 per step; the diagonal emission scaling
is a projective isometry), so after the burn-in each chain's state is
proportional to the true alpha: y^k_t = lam_k * alpha_t. The scale factors
cancel via per-sequence telescoping of the linear functional f_t = 1^T u_t
read at the shared boundary el-time 32k-1 from both neighbouring chains:

  log lam_k = log lam_{k-1} + log f(y^k) - log f(y^{k-1})   (same t, same f)

Each sequence's logZ is read from the sink state of the chain containing its
end position: sink captures 1^T u_{len-1} exactly at t == len and is
absorbing afterwards. Serial depth drops from 256 ticks (fwd+bwd midpoint
baseline) to NT = 40 ticks.

Layout per core (128 sequences x 16 chains = 2048 chain-columns):
  2 strands x 3 groups x 342 columns; partitions 0..95 = active labels
  (3 groups x 32), 96..98 = sink row per group; psum rows 99..101 =
  per-group column sums (ones-columns of the stationary operand).
  Strand 0 = chains 0..7 (TT on Vector), strand 1 = chains 8..15 (TT on
  GpSimd), so the two serial MM->TT->MM dependency chains overlap across
  engines. Snapshots (ACT-Ln of psum colsum rows) at ticks 8/32/40; final
  sink rows Ln'd from the last state. All emissions carry e^{-CSHIFT}; host
  adds CSHIFT*len back and does the gold-path gathers and telescoping.
"""

import numpy as np
import ml_dtypes

B, T, L = 1024, 512, 32
NCORES = 8
BPC = B // NCORES        # 128 sequences per core
K = 32                   # parallel chains per sequence
SEG = T // K             # el-times owned per chain
BURN = 2                 # burn-in ticks (direction convergence)
NT = BURN + SEG          # ticks per chain
NS = 4                   # strands (independent MM->TT dependency chains)
G = 3                    # label groups per strand
NCOL = 342               # columns per group (1024 pairs = 3*342 - 2)
PPS = K * BPC // NS      # 1024 (k,b) pairs per strand
NACT = 96
NPART = 99
MOUT = 102
CSHIFT = 4.5
TAU_SNAP = (BURN, SEG, NT)   # colsum snapshot ticks

_prog_cache = {}
last_result = None       # BassKernelResults of the most recent run (for test.py)


def _build_program():
    import concourse.bacc as bacc
    import concourse.tile as tile
    from concourse import mybir

    f32 = mybir.dt.float32
    bf16 = mybir.dt.bfloat16
    AF = mybir.ActivationFunctionType

    nc = bacc.Bacc("TRN2", target_bir_lowering=False, debug=False, num_devices=NCORES)
    els = [
        nc.dram_tensor(f"el{s}", [NPART, NT + 1, NCOL], bf16, kind="ExternalInput")
        for s in range(NS)
    ]
    wf = nc.dram_tensor("wf", [NPART, MOUT], bf16, kind="ExternalInput")
    outs = [
        nc.dram_tensor(f"res{s}", [21, NCOL], f32, kind="ExternalOutput")
        for s in range(NS)
    ]

    # el DMA chunk boundaries: small first chunk so tick 1 starts early
    CUTS = [0, 2, 4, 7, 11, 15, NT + 1]

    with tile.TileContext(nc) as tc:
        with (
            tc.tile_pool(name="big", bufs=1) as big,
            tc.tile_pool(name="consts", bufs=1) as consts,
            tc.tile_pool(name="u0", bufs=3) as up0,
            tc.tile_pool(name="u1", bufs=3) as up1,
            tc.tile_pool(name="u2", bufs=3) as up2,
            tc.tile_pool(name="u3", bufs=3) as up3,
            tc.tile_pool(name="fin", bufs=1) as fin,
            tc.tile_pool(name="ps0", bufs=2, space="PSUM") as psp0,
            tc.tile_pool(name="ps1", bufs=2, space="PSUM") as psp1,
            tc.tile_pool(name="ps2", bufs=2, space="PSUM") as psp2,
            tc.tile_pool(name="ps3", bufs=2, space="PSUM") as psp3,
        ):
            wf_sb = consts.tile([NPART, MOUT], bf16)
            nc.sync.dma_start(out=wf_sb[:], in_=wf[:])

            el_sb = [big.tile([NPART, NT + 1, NCOL], bf16, tag=f"el{s}", name=f"el_sb{s}") for s in range(NS)]
            dma_engs = (nc.sync, nc.scalar)
            di = 0
            for ci in range(len(CUTS) - 1):
                t0, t1 = CUTS[ci], CUTS[ci + 1]
                for s in range(NS):
                    dma_engs[di % 2].dma_start(
                        out=el_sb[s][:, t0:t1, :], in_=els[s][:, t0:t1, :]
                    )
                    di += 1

            snaps = [
                [fin.tile([6, NCOL], f32, tag=f"sn{s}{j}", name=f"sn{s}{j}") for j in range(3)]
                for s in range(NS)
            ]
            snks = [fin.tile([3, NCOL], f32, tag=f"snk{s}", name=f"snk{s}") for s in range(NS)]
            upools = (up0, up1, up2, up3)
            pspools = (psp0, psp1, psp2, psp3)
            tt_eng = (nc.vector,) * NS

            u = [el_sb[s][:, 0, :] for s in range(NS)]
            for tau in range(1, NT + 1):
                for s in range(NS):
                    ps = pspools[s].tile([MOUT, NCOL], f32, tag=f"ps{s}", name=f"ps{s}")
                    nc.tensor.matmul(ps[:], wf_sb[:], u[s], start=True, stop=True)
                    if tau in TAU_SNAP:
                        j = TAU_SNAP.index(tau)
                        nc.scalar.activation(snaps[s][j][:], ps[NACT:MOUT, :], AF.Ln)
                        if j < 2:
                            nc.sync.dma_start(
                                out=outs[s][6 * j : 6 * j + 6, :], in_=snaps[s][j][:]
                            )
                    un = upools[s].tile([NPART, NCOL], bf16, tag=f"u{s}", name=f"un{s}")
                    tt_eng[s].tensor_mul(un[:], ps[0:NPART, :], el_sb[s][:, tau, :])
                    u[s] = un[:]

            for s in range(NS):
                nc.scalar.activation(snks[s][:], u[s][NACT:NPART, :], AF.Ln)
                nc.sync.dma_start(out=outs[s][12:18, :], in_=snaps[s][2][:])
                nc.scalar.dma_start(out=outs[s][18:21, :], in_=snks[s][:])

    nc.compile()
    return nc


def _host_prep(logits, trans, labels, seq_lens):
    logits = np.ascontiguousarray(np.asarray(logits), dtype=np.float32)
    trans = np.asarray(trans, dtype=np.float32)
    labels = np.asarray(labels)
    lens = np.clip(np.asarray(seq_lens), 1, T).astype(np.int64)

    # ---- gold path score (host: index gathers over small inputs) ----
    tmask = np.arange(T)[None, :] < lens[:, None]
    unary = np.take_along_axis(logits, labels[..., None].astype(np.int64), axis=2)[..., 0]
    gp = (unary * tmask).sum(1) + (trans[labels[:, :-1], labels[:, 1:]] * tmask[:, 1:]).sum(1)

    # ---- full emission tables over el-time 0..T ----
    act = np.where(tmask[:, :, None], np.exp(logits - CSHIFT), 0.0).astype(np.float32)
    act = np.concatenate([act, np.zeros((B, 1, L), np.float32)], axis=1)  # [B,513,L]
    snk = (np.arange(T + 1)[None, :] >= lens[:, None]).astype(np.float32)  # [B,513]

    # chain start offsets and per-tick el-times
    a_k = np.maximum(np.arange(K) * SEG - BURN, 0)           # [K]
    times = a_k[:, None] + np.arange(NT + 1)[None, :]        # [K, NT+1]

    bf = ml_dtypes.bfloat16
    el_cores = []
    for core in range(NCORES):
        b0 = core * BPC
        # pair p = k*BPC + b_local ; strand = p // PPS ; idx = p % PPS
        bidx = b0 + np.tile(np.arange(BPC), K).reshape(K, BPC)       # [K,B_l]
        ap = act[bidx[:, :, None], times[:, None, :], :]             # [K,B_l,NT+1,L]
        sp = snk[bidx[:, :, None], times[:, None, :]]                # [K,B_l,NT+1]
        ap = ap.reshape(K * BPC, NT + 1, L)
        sp = sp.reshape(K * BPC, NT + 1)
        per_strand = []
        for s in range(NS):
            a_s = ap[s * PPS : (s + 1) * PPS]                        # [1024,NT+1,L]
            s_s = sp[s * PPS : (s + 1) * PPS]
            buf = np.zeros((G, 32, NT + 1, NCOL), np.float32)
            sbuf = np.zeros((G, NT + 1, NCOL), np.float32)
            for g in range(G):
                i0 = g * NCOL
                ncols = min(NCOL, PPS - i0)
                buf[g, :, :, :ncols] = a_s[i0 : i0 + ncols].transpose(2, 1, 0)
                sbuf[g, :, :ncols] = s_s[i0 : i0 + ncols].T
                if ncols < NCOL:
                    sbuf[g, :, ncols:] = 1.0  # pad: pure-sink column
            full = np.concatenate([buf.reshape(NACT, NT + 1, NCOL), sbuf], axis=0)
            per_strand.append(np.ascontiguousarray(full.astype(bf)))
        el_cores.append(per_strand)

    # ---- stationary operator: E blocks + sink + colsum columns ----
    E = np.exp(trans).astype(np.float32)
    Wf = np.zeros((NPART, MOUT), np.float32)
    for g in range(G):
        a, sk, cs = 32 * g, NACT + g, NPART + g
        Wf[a : a + 32, a : a + 32] = E
        Wf[a : a + 32, sk] = 1.0
        Wf[sk, sk] = 1.0
        Wf[a : a + 32, cs] = 1.0
        Wf[sk, cs] = 1.0
    return gp, lens, el_cores, Wf.astype(bf)


def _log(msg):
    import time as _t

    print(f"[kernel {_t.strftime('%H:%M:%S')}] {msg}", flush=True)


def kernel(logits, trans, labels, seq_lens):
    global last_result
    from concourse.bass_utils import run_bass_kernel_spmd

    _log("host prep start")
    gp, lens, el_cores, Wf = _host_prep(logits, trans, labels, seq_lens)
    _log("host prep done")

    if "nc" not in _prog_cache:
        _prog_cache["nc"] = _build_program()
        _log("program built")
    nc = _prog_cache["nc"]

    in_maps = [
        {"wf": Wf, **{f"el{s}": el_cores[i][s] for s in range(NS)}}
        for i in range(NCORES)
    ]
    r = run_bass_kernel_spmd(nc, in_maps, core_ids=list(range(NCORES)))
    last_result = r
    _log("device run done")

    # ---- unshard: per (k, b) snapshots and sink reads ----
    lnS = np.zeros((K, B), np.float64)   # colsum at tick BURN   (chain start)
    lnM = np.zeros((K, B), np.float64)   # colsum at tick SEG    (chain-0 end)
    lnE = np.zeros((K, B), np.float64)   # colsum at tick NT     (chain end)
    lnK = np.zeros((K, B), np.float64)   # final sink rows
    for core in range(NCORES):
        b0 = core * BPC
        for s in range(NS):
            res = r.results[core][f"res{s}"].astype(np.float64)  # [21, NCOL]
            idx = np.arange(PPS)
            g, c = idx // NCOL, idx % NCOL
            p = s * PPS + idx
            k, bl = p // BPC, p % BPC
            lnS[k, b0 + bl] = res[3 + g, c]
            lnM[k, b0 + bl] = res[9 + g, c]
            lnE[k, b0 + bl] = res[15 + g, c]
            lnK[k, b0 + bl] = res[18 + g, c]

    # ---- telescoping: log lam_k relative to the exact chain 0 ----
    loglam = np.zeros((K, B), np.float64)
    prev_end = lnM[0]                      # chain 0 boundary read at tick SEG
    for k in range(1, K):
        loglam[k] = loglam[k - 1] + lnS[k] - prev_end
        prev_end = lnE[k]

    kb = np.clip((lens - 1) // SEG, 0, K - 1)
    ar = np.arange(B)
    logZ = lnK[kb, ar] - loglam[kb, ar] + CSHIFT * lens.astype(np.float64)
    return (gp - logZ).astype(np.float32)


# revision 12
# speedup vs baseline: 1.0536x; 1.0536x over previous
"""CRF log-likelihood kernel for Trainium2 (Bass/Tile), 8-core data parallel.

out[b] = gold_path_score(b) - logZ(b)

logZ via K=16 parallel forward chains in the exp domain. Chain k owns
el-times (32k, 32k+32] and starts BURN=8 ticks early at a_k = 32k-8 with an
arbitrary positive init (the el slice at a_k). Products of positive matrices
contract in the Hilbert projective metric (Birkhoff coefficient of
E = exp(trans) is <= tanh(1/2) ~ 0.46 per step; the diagonal emission scaling
is a projective isometry), so after the burn-in each chain's state is
proportional to the true alpha: y^k_t = lam_k * alpha_t. The scale factors
cancel via per-sequence telescoping of the linear functional f_t = 1^T u_t
read at the shared boundary el-time 32k-1 from both neighbouring chains:

  log lam_k = log lam_{k-1} + log f(y^k) - log f(y^{k-1})   (same t, same f)

Each sequence's logZ is read from the sink state of the chain containing its
end position: sink captures 1^T u_{len-1} exactly at t == len and is
absorbing afterwards. Serial depth drops from 256 ticks (fwd+bwd midpoint
baseline) to NT = 40 ticks.

Layout per core (128 sequences x 16 chains = 2048 chain-columns):
  2 strands x 3 groups x 342 columns; partitions 0..95 = active labels
  (3 groups x 32), 96..98 = sink row per group; psum rows 99..101 =
  per-group column sums (ones-columns of the stationary operand).
  Strand 0 = chains 0..7 (TT on Vector), strand 1 = chains 8..15 (TT on
  GpSimd), so the two serial MM->TT->MM dependency chains overlap across
  engines. Snapshots (ACT-Ln of psum colsum rows) at ticks 8/32/40; final
  sink rows Ln'd from the last state. All emissions carry e^{-CSHIFT}; host
  adds CSHIFT*len back and does the gold-path gathers and telescoping.
"""

import numpy as np
import ml_dtypes

B, T, L = 1024, 512, 32
NCORES = 8
BPC = B // NCORES        # 128 sequences per core
K = 32                   # parallel chains per sequence
SEG = T // K             # el-times owned per chain
BURN = 2                 # burn-in ticks (direction convergence)
NT = BURN + SEG          # ticks per chain
NS = 4                   # strands (independent MM->TT dependency chains)
G = 3                    # label groups per strand
NCOL = 342               # columns per group (1024 pairs = 3*342 - 2)
PPS = K * BPC // NS      # 1024 (k,b) pairs per strand
NACT = 96
NPART = 99
MOUT = 102
CSHIFT = 4.5
TAU_SNAP = (BURN, SEG, NT)   # colsum snapshot ticks

_prog_cache = {}
last_result = None       # BassKernelResults of the most recent run (for test.py)


def _build_program():
    import concourse.bacc as bacc
    import concourse.tile as tile
    from concourse import mybir

    f32 = mybir.dt.float32
    bf16 = mybir.dt.bfloat16
    AF = mybir.ActivationFunctionType

    nc = bacc.Bacc("TRN2", target_bir_lowering=False, debug=False, num_devices=NCORES)
    els = [
        nc.dram_tensor(f"el{s}", [NPART, NT + 1, NCOL], bf16, kind="ExternalInput")
        for s in range(NS)
    ]
    wf = nc.dram_tensor("wf", [NPART, MOUT], bf16, kind="ExternalInput")
    outs = [
        nc.dram_tensor(f"res{s}", [21, NCOL], f32, kind="ExternalOutput")
        for s in range(NS)
    ]

    # el DMA chunk boundaries: small first chunk so tick 1 starts early
    CUTS = [0, 2, 6, 12, NT + 1]

    with tile.TileContext(nc) as tc:
        with (
            tc.tile_pool(name="big", bufs=1) as big,
            tc.tile_pool(name="consts", bufs=1) as consts,
            tc.tile_pool(name="u0", bufs=3) as up0,
            tc.tile_pool(name="u1", bufs=3) as up1,
            tc.tile_pool(name="u2", bufs=3) as up2,
            tc.tile_pool(name="u3", bufs=3) as up3,
            tc.tile_pool(name="fin", bufs=1) as fin,
            tc.tile_pool(name="ps0", bufs=2, space="PSUM") as psp0,
            tc.tile_pool(name="ps1", bufs=2, space="PSUM") as psp1,
            tc.tile_pool(name="ps2", bufs=2, space="PSUM") as psp2,
            tc.tile_pool(name="ps3", bufs=2, space="PSUM") as psp3,
        ):
            wf_sb = consts.tile([NPART, MOUT], bf16)
            nc.sync.dma_start(out=wf_sb[:], in_=wf[:])

            el_sb = [big.tile([NPART, NT + 1, NCOL], bf16, tag=f"el{s}", name=f"el_sb{s}") for s in range(NS)]
            dma_engs = (nc.sync, nc.scalar)
            di = 0
            for ci in range(len(CUTS) - 1):
                t0, t1 = CUTS[ci], CUTS[ci + 1]
                for s in range(NS):
                    dma_engs[di % 2].dma_start(
                        out=el_sb[s][:, t0:t1, :], in_=els[s][:, t0:t1, :]
                    )
                    di += 1

            snaps = [
                [fin.tile([6, NCOL], f32, tag=f"sn{s}{j}", name=f"sn{s}{j}") for j in range(3)]
                for s in range(NS)
            ]
            snks = [fin.tile([3, NCOL], f32, tag=f"snk{s}", name=f"snk{s}") for s in range(NS)]
            upools = (up0, up1, up2, up3)
            pspools = (psp0, psp1, psp2, psp3)
            tt_eng = (nc.vector,) * NS

            u = [el_sb[s][:, 0, :] for s in range(NS)]
            for tau in range(1, NT + 1):
                for s in range(NS):
                    ps = pspools[s].tile([MOUT, NCOL], f32, tag=f"ps{s}", name=f"ps{s}")
                    nc.tensor.matmul(ps[:], wf_sb[:], u[s], start=True, stop=True)
                    if tau in TAU_SNAP:
                        j = TAU_SNAP.index(tau)
                        nc.scalar.activation(snaps[s][j][:], ps[NACT:MOUT, :], AF.Ln)
                        if j < 2:
                            nc.sync.dma_start(
                                out=outs[s][6 * j : 6 * j + 6, :], in_=snaps[s][j][:]
                            )
                    un = upools[s].tile([NPART, NCOL], bf16, tag=f"u{s}", name=f"un{s}")
                    tt_eng[s].tensor_mul(un[:], ps[0:NPART, :], el_sb[s][:, tau, :])
                    u[s] = un[:]

            for s in range(NS):
                nc.scalar.activation(snks[s][:], u[s][NACT:NPART, :], AF.Ln)
                nc.sync.dma_start(out=outs[s][12:18, :], in_=snaps[s][2][:])
                nc.scalar.dma_start(out=outs[s][18:21, :], in_=snks[s][:])

    nc.compile()
    return nc


def _host_prep(logits, trans, labels, seq_lens):
    logits = np.ascontiguousarray(np.asarray(logits), dtype=np.float32)
    trans = np.asarray(trans, dtype=np.float32)
    labels = np.asarray(labels)
    lens = np.clip(np.asarray(seq_lens), 1, T).astype(np.int64)

    # ---- gold path score (host: index gathers over small inputs) ----
    tmask = np.arange(T)[None, :] < lens[:, None]
    unary = np.take_along_axis(logits, labels[..., None].astype(np.int64), axis=2)[..., 0]
    gp = (unary * tmask).sum(1) + (trans[labels[:, :-1], labels[:, 1:]] * tmask[:, 1:]).sum(1)

    # ---- full emission tables over el-time 0..T ----
    act = np.where(tmask[:, :, None], np.exp(logits - CSHIFT), 0.0).astype(np.float32)
    act = np.concatenate([act, np.zeros((B, 1, L), np.float32)], axis=1)  # [B,513,L]
    snk = (np.arange(T + 1)[None, :] >= lens[:, None]).astype(np.float32)  # [B,513]

    # chain start offsets and per-tick el-times
    a_k = np.maximum(np.arange(K) * SEG - BURN, 0)           # [K]
    times = a_k[:, None] + np.arange(NT + 1)[None, :]        # [K, NT+1]

    bf = ml_dtypes.bfloat16
    el_cores = []
    for core in range(NCORES):
        b0 = core * BPC
        # pair p = k*BPC + b_local ; strand = p // PPS ; idx = p % PPS
        bidx = b0 + np.tile(np.arange(BPC), K).reshape(K, BPC)       # [K,B_l]
        ap = act[bidx[:, :, None], times[:, None, :], :]             # [K,B_l,NT+1,L]
        sp = snk[bidx[:, :, None], times[:, None, :]]                # [K,B_l,NT+1]
        ap = ap.reshape(K * BPC, NT + 1, L)
        sp = sp.reshape(K * BPC, NT + 1)
        per_strand = []
        for s in range(NS):
            a_s = ap[s * PPS : (s + 1) * PPS]                        # [1024,NT+1,L]
            s_s = sp[s * PPS : (s + 1) * PPS]
            buf = np.zeros((G, 32, NT + 1, NCOL), np.float32)
            sbuf = np.zeros((G, NT + 1, NCOL), np.float32)
            for g in range(G):
                i0 = g * NCOL
                ncols = min(NCOL, PPS - i0)
                buf[g, :, :, :ncols] = a_s[i0 : i0 + ncols].transpose(2, 1, 0)
                sbuf[g, :, :ncols] = s_s[i0 : i0 + ncols].T
                if ncols < NCOL:
                    sbuf[g, :, ncols:] = 1.0  # pad: pure-sink column
            full = np.concatenate([buf.reshape(NACT, NT + 1, NCOL), sbuf], axis=0)
            per_strand.append(np.ascontiguousarray(full.astype(bf)))
        el_cores.append(per_strand)

    # ---- stationary operator: E blocks + sink + colsum columns ----
    E = np.exp(trans).astype(np.float32)
    Wf = np.zeros((NPART, MOUT), np.float32)
    for g in range(G):
        a, sk, cs = 32 * g, NACT + g, NPART + g
        Wf[a : a + 32, a : a + 32] = E
        Wf[a : a + 32, sk] = 1.0
        Wf[sk, sk] = 1.0
        Wf[a : a + 32, cs] = 1.0
        Wf[sk, cs] = 1.0
    return gp, lens, el_cores, Wf.astype(bf)


def _log(msg):
    import time as _t

    print(f"[kernel {_t.strftime('%H:%M:%S')}] {msg}", flush=True)


def kernel(logits, trans, labels, seq_lens):
    global last_result
    from concourse.bass_utils import run_bass_kernel_spmd

    _log("host prep start")
    gp, lens, el_cores, Wf = _host_prep(logits, trans, labels, seq_lens)
    _log("host prep done")

    if "nc" not in _prog_cache:
        _prog_cache["nc"] = _build_program()
        _log("program built")
    nc = _prog_cache["nc"]

    in_maps = [
        {"wf": Wf, **{f"el{s}": el_cores[i][s] for s in range(NS)}}
        for i in range(NCORES)
    ]
    r = run_bass_kernel_spmd(nc, in_maps, core_ids=list(range(NCORES)))
    last_result = r
    _log("device run done")

    # ---- unshard: per (k, b) snapshots and sink reads ----
    lnS = np.zeros((K, B), np.float64)   # colsum at tick BURN   (chain start)
    lnM = np.zeros((K, B), np.float64)   # colsum at tick SEG    (chain-0 end)
    lnE = np.zeros((K, B), np.float64)   # colsum at tick NT     (chain end)
    lnK = np.zeros((K, B), np.float64)   # final sink rows
    for core in range(NCORES):
        b0 = core * BPC
        for s in range(NS):
            res = r.results[core][f"res{s}"].astype(np.float64)  # [21, NCOL]
            idx = np.arange(PPS)
            g, c = idx // NCOL, idx % NCOL
            p = s * PPS + idx
            k, bl = p // BPC, p % BPC
            lnS[k, b0 + bl] = res[3 + g, c]
            lnM[k, b0 + bl] = res[9 + g, c]
            lnE[k, b0 + bl] = res[15 + g, c]
            lnK[k, b0 + bl] = res[18 + g, c]

    # ---- telescoping: log lam_k relative to the exact chain 0 ----
    loglam = np.zeros((K, B), np.float64)
    prev_end = lnM[0]                      # chain 0 boundary read at tick SEG
    for k in range(1, K):
        loglam[k] = loglam[k - 1] + lnS[k] - prev_end
        prev_end = lnE[k]

    kb = np.clip((lens - 1) // SEG, 0, K - 1)
    ar = np.arange(B)
    logZ = lnK[kb, ar] - loglam[kb, ar] + CSHIFT * lens.astype(np.float64)
    return (gp - logZ).astype(np.float32)


# revision 13
# speedup vs baseline: 1.0666x; 1.0124x over previous
"""CRF log-likelihood kernel for Trainium2 (Bass/Tile), 8-core data parallel.

out[b] = gold_path_score(b) - logZ(b)

logZ via K=16 parallel forward chains in the exp domain. Chain k owns
el-times (32k, 32k+32] and starts BURN=8 ticks early at a_k = 32k-8 with an
arbitrary positive init (the el slice at a_k). Products of positive matrices
contract in the Hilbert projective metric (Birkhoff coefficient of
E = exp(trans) is <= tanh(1/2) ~ 0.46 per step; the diagonal emission scaling
is a projective isometry), so after the burn-in each chain's state is
proportional to the true alpha: y^k_t = lam_k * alpha_t. The scale factors
cancel via per-sequence telescoping of the linear functional f_t = 1^T u_t
read at the shared boundary el-time 32k-1 from both neighbouring chains:

  log lam_k = log lam_{k-1} + log f(y^k) - log f(y^{k-1})   (same t, same f)

Each sequence's logZ is read from the sink state of the chain containing its
end position: sink captures 1^T u_{len-1} exactly at t == len and is
absorbing afterwards. Serial depth drops from 256 ticks (fwd+bwd midpoint
baseline) to NT = 40 ticks.

Layout per core (128 sequences x 16 chains = 2048 chain-columns):
  2 strands x 3 groups x 342 columns; partitions 0..95 = active labels
  (3 groups x 32), 96..98 = sink row per group; psum rows 99..101 =
  per-group column sums (ones-columns of the stationary operand).
  Strand 0 = chains 0..7 (TT on Vector), strand 1 = chains 8..15 (TT on
  GpSimd), so the two serial MM->TT->MM dependency chains overlap across
  engines. Snapshots (ACT-Ln of psum colsum rows) at ticks 8/32/40; final
  sink rows Ln'd from the last state. All emissions carry e^{-CSHIFT}; host
  adds CSHIFT*len back and does the gold-path gathers and telescoping.
"""

import numpy as np
import ml_dtypes

B, T, L = 1024, 512, 32
NCORES = 8
BPC = B // NCORES        # 128 sequences per core
K = 32                   # parallel chains per sequence
SEG = T // K             # el-times owned per chain
BURN = 2                 # burn-in ticks (direction convergence)
NT = BURN + SEG          # ticks per chain
NS = 4                   # strands (independent MM->TT dependency chains)
G = 3                    # label groups per strand
NCOL = 342               # columns per group (1024 pairs = 3*342 - 2)
PPS = K * BPC // NS      # 1024 (k,b) pairs per strand
NACT = 96
NPART = 99
MOUT = 102
CSHIFT = 4.5
TAU_SNAP = (BURN, SEG, NT)   # colsum snapshot ticks

_prog_cache = {}
last_result = None       # BassKernelResults of the most recent run (for test.py)


def _build_program():
    import concourse.bacc as bacc
    import concourse.tile as tile
    from concourse import mybir

    f32 = mybir.dt.float32
    bf16 = mybir.dt.bfloat16
    AF = mybir.ActivationFunctionType

    nc = bacc.Bacc("TRN2", target_bir_lowering=False, debug=False, num_devices=NCORES)
    els = [
        nc.dram_tensor(f"el{s}", [NPART, NT + 1, NCOL], bf16, kind="ExternalInput")
        for s in range(NS)
    ]
    wf = nc.dram_tensor("wf", [NPART, MOUT], bf16, kind="ExternalInput")
    outs = [
        nc.dram_tensor(f"res{s}", [NPART, NCOL], f32, kind="ExternalOutput")
        for s in range(NS)
    ]

    # el DMA chunk boundaries: small first chunk so tick 1 starts early
    CUTS = [0, 2, 6, 12, NT + 1]

    with tile.TileContext(nc) as tc:
        with (
            tc.tile_pool(name="big", bufs=1) as big,
            tc.tile_pool(name="consts", bufs=1) as consts,
            tc.tile_pool(name="u0", bufs=3) as up0,
            tc.tile_pool(name="u1", bufs=3) as up1,
            tc.tile_pool(name="u2", bufs=3) as up2,
            tc.tile_pool(name="u3", bufs=3) as up3,
            tc.tile_pool(name="fin", bufs=1) as fin,
            tc.tile_pool(name="ps0", bufs=2, space="PSUM") as psp0,
            tc.tile_pool(name="ps1", bufs=2, space="PSUM") as psp1,
            tc.tile_pool(name="ps2", bufs=2, space="PSUM") as psp2,
            tc.tile_pool(name="ps3", bufs=2, space="PSUM") as psp3,
        ):
            wf_sb = consts.tile([NPART, MOUT], bf16)
            nc.sync.dma_start(out=wf_sb[:], in_=wf[:])

            el_sb = [big.tile([NPART, NT + 1, NCOL], bf16, tag=f"el{s}", name=f"el_sb{s}") for s in range(NS)]
            dma_engs = (nc.sync, nc.scalar)
            di = 0
            for ci in range(len(CUTS) - 1):
                t0, t1 = CUTS[ci], CUTS[ci + 1]
                for s in range(NS):
                    dma_engs[di % 2].dma_start(
                        out=el_sb[s][:, t0:t1, :], in_=els[s][:, t0:t1, :]
                    )
                    di += 1

            res_sb = [
                fin.tile([NPART, NCOL], f32, tag=f"res{s}", name=f"res_sb{s}")
                for s in range(NS)
            ]
            upools = (up0, up1, up2, up3)
            pspools = (psp0, psp1, psp2, psp3)
            tt_eng = (nc.vector,) * NS

            u = [el_sb[s][:, 0, :] for s in range(NS)]
            for tau in range(1, NT + 1):
                for s in range(NS):
                    ps = pspools[s].tile([MOUT, NCOL], f32, tag=f"ps{s}", name=f"ps{s}")
                    nc.tensor.matmul(ps[:], wf_sb[:], u[s], start=True, stop=True)
                    if tau in TAU_SNAP:
                        j = TAU_SNAP.index(tau)
                        nc.scalar.activation(
                            res_sb[s][32 * j : 32 * j + 6, :], ps[NACT:MOUT, :], AF.Ln
                        )
                    un = upools[s].tile([NPART, NCOL], bf16, tag=f"u{s}", name=f"un{s}")
                    tt_eng[s].tensor_mul(un[:], ps[0:NPART, :], el_sb[s][:, tau, :])
                    u[s] = un[:]

            for s in range(NS):
                nc.scalar.activation(res_sb[s][NACT:NPART, :], u[s][NACT:NPART, :], AF.Ln)
                eng = nc.sync if s % 2 == 0 else nc.scalar
                eng.dma_start(out=outs[s][:], in_=res_sb[s][:])

    nc.compile()
    return nc


def _host_prep(logits, trans, labels, seq_lens):
    logits = np.ascontiguousarray(np.asarray(logits), dtype=np.float32)
    trans = np.asarray(trans, dtype=np.float32)
    labels = np.asarray(labels)
    lens = np.clip(np.asarray(seq_lens), 1, T).astype(np.int64)

    # ---- gold path score (host: index gathers over small inputs) ----
    tmask = np.arange(T)[None, :] < lens[:, None]
    unary = np.take_along_axis(logits, labels[..., None].astype(np.int64), axis=2)[..., 0]
    gp = (unary * tmask).sum(1) + (trans[labels[:, :-1], labels[:, 1:]] * tmask[:, 1:]).sum(1)

    # ---- full emission tables over el-time 0..T ----
    act = np.where(tmask[:, :, None], np.exp(logits - CSHIFT), 0.0).astype(np.float32)
    act = np.concatenate([act, np.zeros((B, 1, L), np.float32)], axis=1)  # [B,513,L]
    snk = (np.arange(T + 1)[None, :] >= lens[:, None]).astype(np.float32)  # [B,513]

    # chain start offsets and per-tick el-times
    a_k = np.maximum(np.arange(K) * SEG - BURN, 0)           # [K]
    times = a_k[:, None] + np.arange(NT + 1)[None, :]        # [K, NT+1]

    bf = ml_dtypes.bfloat16
    el_cores = []
    for core in range(NCORES):
        b0 = core * BPC
        # pair p = k*BPC + b_local ; strand = p // PPS ; idx = p % PPS
        bidx = b0 + np.tile(np.arange(BPC), K).reshape(K, BPC)       # [K,B_l]
        ap = act[bidx[:, :, None], times[:, None, :], :]             # [K,B_l,NT+1,L]
        sp = snk[bidx[:, :, None], times[:, None, :]]                # [K,B_l,NT+1]
        ap = ap.reshape(K * BPC, NT + 1, L)
        sp = sp.reshape(K * BPC, NT + 1)
        per_strand = []
        for s in range(NS):
            a_s = ap[s * PPS : (s + 1) * PPS]                        # [1024,NT+1,L]
            s_s = sp[s * PPS : (s + 1) * PPS]
            buf = np.zeros((G, 32, NT + 1, NCOL), np.float32)
            sbuf = np.zeros((G, NT + 1, NCOL), np.float32)
            for g in range(G):
                i0 = g * NCOL
                ncols = min(NCOL, PPS - i0)
                buf[g, :, :, :ncols] = a_s[i0 : i0 + ncols].transpose(2, 1, 0)
                sbuf[g, :, :ncols] = s_s[i0 : i0 + ncols].T
                if ncols < NCOL:
                    sbuf[g, :, ncols:] = 1.0  # pad: pure-sink column
            full = np.concatenate([buf.reshape(NACT, NT + 1, NCOL), sbuf], axis=0)
            per_strand.append(np.ascontiguousarray(full.astype(bf)))
        el_cores.append(per_strand)

    # ---- stationary operator: E blocks + sink + colsum columns ----
    E = np.exp(trans).astype(np.float32)
    Wf = np.zeros((NPART, MOUT), np.float32)
    for g in range(G):
        a, sk, cs = 32 * g, NACT + g, NPART + g
        Wf[a : a + 32, a : a + 32] = E
        Wf[a : a + 32, sk] = 1.0
        Wf[sk, sk] = 1.0
        Wf[a : a + 32, cs] = 1.0
        Wf[sk, cs] = 1.0
    return gp, lens, el_cores, Wf.astype(bf)


def _log(msg):
    import time as _t

    print(f"[kernel {_t.strftime('%H:%M:%S')}] {msg}", flush=True)


def kernel(logits, trans, labels, seq_lens):
    global last_result
    from concourse.bass_utils import run_bass_kernel_spmd

    _log("host prep start")
    gp, lens, el_cores, Wf = _host_prep(logits, trans, labels, seq_lens)
    _log("host prep done")

    if "nc" not in _prog_cache:
        _prog_cache["nc"] = _build_program()
        _log("program built")
    nc = _prog_cache["nc"]

    in_maps = [
        {"wf": Wf, **{f"el{s}": el_cores[i][s] for s in range(NS)}}
        for i in range(NCORES)
    ]
    r = run_bass_kernel_spmd(nc, in_maps, core_ids=list(range(NCORES)))
    last_result = r
    _log("device run done")

    # ---- unshard: per (k, b) snapshots and sink reads ----
    lnS = np.zeros((K, B), np.float64)   # colsum at tick BURN   (chain start)
    lnM = np.zeros((K, B), np.float64)   # colsum at tick SEG    (chain-0 end)
    lnE = np.zeros((K, B), np.float64)   # colsum at tick NT     (chain end)
    lnK = np.zeros((K, B), np.float64)   # final sink rows
    for core in range(NCORES):
        b0 = core * BPC
        for s in range(NS):
            res = r.results[core][f"res{s}"].astype(np.float64)  # [NPART, NCOL]
            idx = np.arange(PPS)
            g, c = idx // NCOL, idx % NCOL
            p = s * PPS + idx
            k, bl = p // BPC, p % BPC
            lnS[k, b0 + bl] = res[3 + g, c]
            lnM[k, b0 + bl] = res[35 + g, c]
            lnE[k, b0 + bl] = res[67 + g, c]
            lnK[k, b0 + bl] = res[NACT + g, c]

    # ---- telescoping: log lam_k relative to the exact chain 0 ----
    loglam = np.zeros((K, B), np.float64)
    prev_end = lnM[0]                      # chain 0 boundary read at tick SEG
    for k in range(1, K):
        loglam[k] = loglam[k - 1] + lnS[k] - prev_end
        prev_end = lnE[k]

    kb = np.clip((lens - 1) // SEG, 0, K - 1)
    ar = np.arange(B)
    logZ = lnK[kb, ar] - loglam[kb, ar] + CSHIFT * lens.astype(np.float64)
    return (gp - logZ).astype(np.float32)
